# revision 31
# baseline (speedup 1.0000x reference)
"""Trainium2 Bass kernel for nn_CRFLayer (ragged sequence linear + token repack).

Reference computation:
    logits = embedding @ W.T + b            # [B, S, L]
    pack masked (mask==1) positions left per row -> [B, max_tok, L], zero pad
    pad_mask = arange(max_tok) < token_lens

Strategy (data parallel over batch, 2 rows / core on 8 cores):
  * Host computes gather indices from the mask (cheap metadata) and splits the
    fp32 embedding into bf16 hi + bf16 lo (hi+lo ~ fp32 to ~2^-17 rel).
  * Device gathers only the masked token rows straight into SBUF *transposed*
    (dma_gather transpose=True puts the contraction dim D on partitions), so
    the heavy HBM traffic is halved vs computing all S positions.
  * 16 accumulating bf16 matmuls per 128-token tile compute
    Ehi@[Whi|Wlo] and Elo@Whi into one PSUM tile; DVE folds the halves and a
    host-prebuilt masked bias (b at valid slots, 0 at pads).  Pad slots gather
    an all-zero row, so padding comes out exactly 0.
  * One batched DMA per core writes the packed [n_tiles*128, 32] result.
"""

import math

import numpy as np

B, S, D, L = 16, 2048, 1024, 32
N_CORES = 8
RPC = B // N_CORES  # batch rows per core
P = 128
DC = D // P  # contraction chunks of 128

# tokens per gather instruction (multiple of 128, divides RPC*n_t*128)
G = 384
N_QUEUES = 1
# raw (non-Tile) pipeline with hand-placed semaphores: avoids the Tile
# preamble (~7us) and exit drain/barrier (~11us)
RAW = True

TRACE = False
LAST = {}

_BUILD_CACHE = {}


def _pick_g(total_slots):
    for g in (G, 512, 256, 128):
        if total_slots % g == 0:
            return g
    return 128


def _windows(total_slots):
    """Tapered gather-window sizes: small first windows get the tensor engine
    started early; small last windows shorten the drain tail."""
    assert total_slots % P == 0
    if total_slots >= 6 * 384 and (total_slots - 768) % 384 == 0:
        k = (total_slots - 768) // 384
        return [128, 256] + [384] * k + [256, 128]
    g = _pick_g(total_slots)
    return [g] * (total_slots // g)


def _build_raw(n_t):
    """Raw Bacc pipeline (no TileContext). Same I/O contract as _build.

    Streams: sync loads consts + per-window lo chunks (HWDGE); gpsimd loads
    the mlp ucode then runs the transposed hi gathers back-to-back into
    distinct buffers; PE runs each window's matmuls once its gather + lo
    chunk land, recycling 8 PSUM banks guarded by the DVE's add_sem; DVE
    folds bias; sync writes each window's packed output as it completes.
    """
    from contextlib import ExitStack

    import concourse.mybir as mybir
    from concourse import bacc, library_config

    n_tiles = RPC * n_t
    wins = _windows(n_tiles * P)
    n_w = len(wins)
    starts = [sum(wins[:g]) for g in range(n_w)]  # slot offsets
    n_rows_pad = RPC * S + 1
    icols = sum(w // 16 for w in wins)

    nc = bacc.Bacc(
        "TRN2",
        debug=False,
        enable_asserts=False,
        num_devices=N_CORES,
        num_swdge_queues=1,
    )
    bf16 = mybir.dt.bfloat16
    f32 = mybir.dt.float32

    emb_hi = nc.dram_tensor("emb_hi", [n_rows_pad, D], bf16, kind="ExternalInput")
    lo_packed = nc.dram_tensor(
        "lo_packed", [P * DC * n_tiles * P], bf16, kind="ExternalInput")
    idx = nc.dram_tensor("idx", [P, icols], mybir.dt.int16, kind="ExternalInput")
    wt = nc.dram_tensor("wt", [P, DC, 2 * L], bf16, kind="ExternalInput")
    bmask = nc.dram_tensor("bmask", [P, n_tiles, L], f32, kind="ExternalInput")
    out = nc.dram_tensor("out", [n_tiles * P, L], f32, kind="ExternalOutput")

    PB = 8  # PSUM banks cycled

    with ExitStack() as st:
        idx_t = st.enter_context(
            nc.sbuf_tensor("idx_t", [P, icols], mybir.dt.int16))
        wt_t = st.enter_context(nc.sbuf_tensor("wt_t", [P, DC, 2 * L], bf16))
        bm_t = st.enter_context(nc.sbuf_tensor("bm_t", [P, n_tiles, L], f32))
        lo_t = st.enter_context(nc.sbuf_tensor("lo_t", [P, DC, n_tiles * P], bf16))
        out_t = st.enter_context(nc.sbuf_tensor("out_t", [P, n_tiles, L], f32))
        ghi = [st.enter_context(
            nc.sbuf_tensor(f"ghi{g}", [P, DC, wins[g]], bf16))
            for g in range(n_w)]
        pss = [st.enter_context(
            nc.psum_tensor(f"ps{b}", [P, 2 * L], f32)) for b in range(PB)]
        i_sem = st.enter_context(nc.semaphore("i_sem"))
        w_sem = st.enter_context(nc.semaphore("w_sem"))
        b_sem = st.enter_context(nc.semaphore("b_sem"))
        gs = [st.enter_context(nc.semaphore(f"gs{g}")) for g in range(n_w)]
        ls = [st.enter_context(nc.semaphore(f"ls{g}")) for g in range(n_w)]
        mm_sem = st.enter_context(nc.semaphore("mm_sem"))
        add_sem = st.enter_context(nc.semaphore("add_sem"))
        aux_sem = st.enter_context(nc.semaphore("aux_sem"))
        d_sem = st.enter_context(nc.semaphore("d_sem"))
        out_sem = st.enter_context(nc.semaphore("out_sem"))

        out_view = out.ap().rearrange("(i p) l -> p i l", p=P)

        # --- sync: const + lo loads (per-window [p, c, t] blocks: one big
        # --- contiguous descriptor per partition per window) ---
        nc.sync.dma_start(out=idx_t[:], in_=idx.ap()).then_inc(i_sem, 16)
        nc.sync.dma_start(out=wt_t[:], in_=wt.ap()).then_inc(w_sem, 16)
        for g in range(n_w):
            w = wins[g]
            ts_g = slice(starts[g], starts[g] + w)
            o_flat = P * DC * starts[g]
            src = lo_packed.ap()[o_flat:o_flat + P * DC * w].rearrange(
                "(p c t) -> p c t", p=P, c=DC)
            nc.sync.dma_start(out=lo_t[:, :, ts_g], in_=src).then_inc(ls[g], 16)
            if g == 1:
                nc.sync.dma_start(out=bm_t[:], in_=bmask.ap()).then_inc(b_sem, 16)

        # --- gpsimd: ucode library + transposed hi gathers ---
        nc.gpsimd.load_library(library_config.mlp)
        nc.gpsimd.wait_ge(i_sem, 16)  # idx_t loaded
        ic = 0
        for g in range(n_w):
            gc = wins[g] // 16
            nc.gpsimd.dma_gather(
                out_ap=ghi[g][:],
                in_ap=emb_hi.ap(),
                idxs_ap=idx_t[:, ic:ic + gc],
                num_idxs=wins[g],
                num_idxs_reg=wins[g],
                elem_size=D,
                transpose=True,
                queue_num=0,
            ).then_inc(gs[g], 16)
            ic += gc

        # --- tensor: per window, per 128-token tile ---
        nc.tensor.wait_ge(w_sem, 16)
        for g in range(n_w):
            nc.tensor.wait_ge(gs[g], 16)
            nc.tensor.wait_ge(ls[g], 16)
            for m in range(wins[g] // P):
                i = starts[g] // P + m
                if i >= PB:
                    nc.tensor.wait_ge(add_sem, i - PB + 1)
                ps = pss[i % PB]
                ms = slice(m * P, (m + 1) * P)
                ts_ = slice(i * P, (i + 1) * P)
                for c in range(DC):
                    nc.tensor.matmul(
                        out=ps[:, :],
                        lhsT=ghi[g][:, c, ms],
                        rhs=wt_t[:, c, :],
                        start=(c == 0),
                        stop=False,
                        skip_group_check=True,
                    )
                    mm = nc.tensor.matmul(
                        out=ps[:, 0:L],
                        lhsT=lo_t[:, c, ts_],
                        rhs=wt_t[:, c, 0:L],
                        start=False,
                        stop=(c == DC - 1),
                        skip_group_check=True,
                    )
                mm.then_inc(mm_sem, 1)

        # --- vector: bias fold + half-merge; drain per window so sync can
        # --- stream the output out behind us
        nc.vector.wait_ge(b_sem, 16)  # bm_t loaded
        for g in range(n_w):
            for m in range(wins[g] // P):
                i = starts[g] // P + m
                nc.vector.wait_ge(mm_sem, i + 1)
                ps = pss[i % PB]
                nc.vector.tensor_add(
                    out=out_t[:, i, :], in0=ps[:, 0:L], in1=bm_t[:, i, :]
                ).then_inc(aux_sem, 1)
                # same-engine RAW on out_t needs a completion wait
                nc.vector.wait_ge(aux_sem, i + 1)
                nc.vector.tensor_add(
                    out=out_t[:, i, :], in0=out_t[:, i, :], in1=ps[:, L:2 * L]
                ).then_inc(add_sem, 1)
            nc.vector.drain().then_inc(d_sem, 1)

        # --- sync: per-window packed output writeback ---
        done = 0
        for g in range(n_w):
            t1 = (starts[g] + wins[g]) // P
            nc.sync.wait_ge(d_sem, g + 1)
            nc.sync.dma_start(
                out=out_view[:, done:t1, :], in_=out_t[:, done:t1, :]
            ).then_inc(out_sem, 16)
            done = t1
        nc.sync.wait_ge(out_sem, 16 * n_w)

    nc.compile()
    return nc


def _build(n_t):
    """Build the Bass program for n_t 128-token tiles per batch row.

    Device inputs (per core):
      emb_hi:    [RPC*S + 1, D] bf16, last row zero (pad target)
      idx:       [P, n_gathers * G/16] int16 gather indices (SWDGE wrap)
      lo_packed: [D, n_tiles*P] bf16 pre-packed transposed lo correction
      wt:        [P, DC, 2L] bf16: wt[p,c,:L] = Whi[:,128c+p].T etc.
      bmask:     [P, n_tiles, L] f32: b at valid slots else 0
    Output: out [n_tiles*P, L] f32 (packed logits, row-major per tile).
    """
    import concourse.mybir as mybir
    import concourse.tile as tile
    from concourse import bacc, library_config

    n_tiles = RPC * n_t  # 128-token tiles per core
    n_gathers = n_tiles * P // G
    sub = G // P  # matmul tiles per gather
    n_rows_pad = RPC * S + 1
    gcols = G // 16  # idx columns per gather

    nc = bacc.Bacc(
        "TRN2",
        debug=False,
        enable_asserts=False,
        num_devices=N_CORES,
        num_swdge_queues=N_QUEUES,
    )
    bf16 = mybir.dt.bfloat16
    f32 = mybir.dt.float32

    emb_hi = nc.dram_tensor("emb_hi", [n_rows_pad, D], bf16, kind="ExternalInput")
    lo_packed = nc.dram_tensor("lo_packed", [D, n_tiles * P], bf16, kind="ExternalInput")
    idx = nc.dram_tensor("idx", [P, n_gathers * gcols], mybir.dt.int16, kind="ExternalInput")
    wt = nc.dram_tensor("wt", [P, DC, 2 * L], bf16, kind="ExternalInput")
    bmask = nc.dram_tensor("bmask", [P, n_tiles, L], f32, kind="ExternalInput")
    out = nc.dram_tensor("out", [n_tiles * P, L], f32, kind="ExternalOutput")

    with tile.TileContext(nc) as tc:
        with (
            tc.tile_pool(name="const", bufs=1) as cpool,
            tc.tile_pool(name="gat", bufs=3) as gpool,
            tc.tile_pool(name="ps", bufs=8, space="PSUM") as ppool,
            tc.tile_pool(name="outp", bufs=1) as opool,
        ):
            nc.gpsimd.load_library(library_config.mlp)

            idx_t = cpool.tile([P, n_gathers * gcols], mybir.dt.int16)
            nc.sync.dma_start(out=idx_t[:], in_=idx.ap())
            wt_t = cpool.tile([P, DC, 2 * L], bf16)
            nc.sync.dma_start(out=wt_t[:], in_=wt.ap())
            bm_t = cpool.tile([P, n_tiles, L], f32)
            nc.sync.dma_start(out=bm_t[:], in_=bmask.ap())
            lo_t = cpool.tile([P, DC, n_tiles * P], bf16)
            nc.sync.dma_start(
                out=lo_t[:],
                in_=lo_packed.ap().rearrange("(c p) t -> p c t", p=P),
            )
            out_t = opool.tile([P, n_tiles, L], f32)

            for g in range(n_gathers):
                ghi = gpool.tile([P, DC, G], bf16, tag="ghi")
                islice = idx_t[:, g * gcols:(g + 1) * gcols]
                nc.gpsimd.dma_gather(
                    out_ap=ghi[:],
                    in_ap=emb_hi.ap(),
                    idxs_ap=islice,
                    num_idxs=G,
                    num_idxs_reg=G,
                    elem_size=D,
                    transpose=True,
                    queue_num=0,
                )
                for m in range(sub):
                    i = g * sub + m  # global 128-token tile index
                    ms = slice(m * P, (m + 1) * P)
                    ts_ = slice(i * P, (i + 1) * P)
                    ps = ppool.tile([P, 2 * L], f32)
                    for c in range(DC):
                        # [Ehi@Whi | Ehi@Wlo] in one N=64 moving pass
                        nc.tensor.matmul(
                            out=ps[:],
                            lhsT=ghi[:, c, ms],
                            rhs=wt_t[:, c, :],
                            start=(c == 0),
                            stop=False,
                            skip_group_check=True,
                        )
                        # + Elo@Whi from the host-packed transposed lo stream
                        nc.tensor.matmul(
                            out=ps[:, 0:L],
                            lhsT=lo_t[:, c, ts_],
                            rhs=wt_t[:, c, 0:L],
                            start=False,
                            stop=(c == DC - 1),
                            skip_group_check=True,
                        )
                    nc.vector.tensor_add(
                        out=out_t[:, i, :], in0=ps[:, 0:L], in1=bm_t[:, i, :]
                    )
                    nc.vector.tensor_add(
                        out=out_t[:, i, :], in0=out_t[:, i, :], in1=ps[:, L:2 * L]
                    )

            nc.sync.dma_start(
                out=out.ap().rearrange("(i p) l -> p i l", p=P),
                in_=out_t[:],
            )

    nc.compile()
    return nc


def _get_nc(n_t):
    key = (n_t, RAW)
    if key not in _BUILD_CACHE:
        _BUILD_CACHE[key] = _build_raw(n_t) if RAW else _build(n_t)
    return _BUILD_CACHE[key]


def _prep_inputs(embedding, mask, W, b):
    import ml_dtypes

    bf16 = ml_dtypes.bfloat16
    lens = mask.astype(np.int64).sum(axis=1)
    max_tok = int(lens.max())
    n_t = (max_tok + P - 1) // P
    n_tiles = RPC * n_t
    wins = _windows(n_tiles * P)
    npad = RPC * S  # index of the zero row

    emb2 = np.ascontiguousarray(embedding.reshape(B * S, D))
    ehi = emb2.astype(bf16)
    elo = (emb2 - ehi.astype(np.float32)).astype(bf16)

    Whi = W.astype(bf16)
    Wlo = (W - Whi.astype(np.float32)).astype(bf16)
    wt_host = np.empty((P, DC, 2 * L), dtype=bf16)
    wt_host[:, :, :L] = Whi.T.reshape(DC, P, L).transpose(1, 0, 2)
    wt_host[:, :, L:] = Wlo.T.reshape(DC, P, L).transpose(1, 0, 2)

    zrow = np.zeros((1, D), dtype=bf16)
    in_maps = []
    for k in range(N_CORES):
        r0 = k * RPC * S
        ehi_k = np.concatenate([ehi[r0:r0 + RPC * S], zrow], axis=0)
        elo_k = np.concatenate([elo[r0:r0 + RPC * S], zrow], axis=0)

        vals = np.full((n_tiles * P,), npad, dtype=np.int16)
        for r in range(RPC):
            pos = np.nonzero(mask[k * RPC + r])[0]
            lr = len(pos)
            o = r * n_t * P
            vals[o:o + lr] = (pos + r * S).astype(np.int16)
        valid = vals != npad
        # lo correction stream: host-packed + transposed, one contiguous
        # [P, DC, w] block per gather window
        packed = elo_k[vals].reshape(-1, DC, P)  # [slots, c, p]
        blocks, o = [], 0
        for w in wins:
            blocks.append(packed[o:o + w].transpose(2, 1, 0).ravel())
            o += w
        lo_packed = np.ascontiguousarray(np.concatenate(blocks))
        # SWDGE idx wrap per window: slot j -> partition j%16, col j//16,
        # replicated into every 16-partition group (each Q7 cpu of the serving
        # queue reads its own group).
        blocks, o = [], 0
        for w in wins:
            blocks.append(vals[o:o + w].reshape(w // 16, 16).T)
            o += w
        idx16 = np.concatenate(blocks, axis=1)
        idx_host = np.tile(idx16, (P // 16, 1))
        bm_host = (
            valid.reshape(n_tiles, P).T[:, :, None].astype(np.float32)
            * b.astype(np.float32)[None, None, :]
        )
        in_maps.append(
            dict(emb_hi=ehi_k, lo_packed=lo_packed, idx=idx_host,
                 wt=wt_host, bmask=np.ascontiguousarray(bm_host))
        )
    return in_maps, lens, max_tok, n_t


def kernel(embedding, mask, W, b):
    from concourse.bass_utils import run_bass_kernel_spmd

    embedding = np.asarray(embedding, dtype=np.float32)
    mask = np.asarray(mask)
    W = np.asarray(W, dtype=np.float32)
    b = np.asarray(b, dtype=np.float32)

    in_maps, lens, max_tok, n_t = _prep_inputs(embedding, mask, W, b)
    nc = _get_nc(n_t)

    res = run_bass_kernel_spmd(
        nc, in_maps, core_ids=list(range(N_CORES)), trace=TRACE
    )
    LAST["results"] = res

    outs = [r["out"].reshape(RPC, n_t * P, L) for r in res.results]
    tok_logits = np.concatenate(outs, axis=0)[:, :max_tok, :].astype(np.float32)
    pad_mask = np.arange(max_tok)[None, :] < lens[:, None]
    return tok_logits, pad_mask


# revision 35
# speedup vs baseline: 2576338.6254x; 2576338.6254x over previous
"""Trainium2 Bass kernel for nn_CRFLayer (ragged sequence linear + token repack).

Reference computation:
    logits = embedding @ W.T + b            # [B, S, L]
    pack masked (mask==1) positions left per row -> [B, max_tok, L], zero pad
    pad_mask = arange(max_tok) < token_lens

Strategy (data parallel over batch, 2 rows / core on 8 cores; raw Bacc
pipeline with hand-placed semaphores):
  * Host computes gather indices from the mask (cheap metadata) and splits the
    fp32 embedding into bf16 hi + lo halves (hi+lo ~ fp32 to ~1e-5 rel).
  * The device gathers only the masked token rows of the hi half straight
    into SBUF *transposed* (dma_gather transpose=True puts the contraction
    dim D on partitions - the only cheap transposition path for a gather),
    so the heavy HBM traffic is roughly halved vs computing all S positions.
    Gather windows are tapered (128/256 first and last) so the tensor engine
    starts early and the drain tail is short.
  * The lo correction stream is pre-packed + transposed on the host and
    streamed densely over HWDGE in per-window contiguous blocks.
  * Per 128-token tile: 8 accumulating bf16 matmuls compute Ehi@[Whi|Wlo]
    (N=64 moving) plus 8 for Elo@Whi into one PSUM tile; the vector engine
    folds the two halves and a host-prebuilt masked bias (b at valid slots,
    0 at pads).  Pad slots gather an all-zero row, so padding is exactly 0.
  * Each window's packed [w, 32] output streams out as soon as its adds
    drain; host concatenates the 8 core outputs and trims to max_tok.
"""

import numpy as np

B, S, D, L = 16, 2048, 1024, 32
N_CORES = 8
RPC = B // N_CORES  # batch rows per core
P = 128
DC = D // P  # contraction chunks of 128

# tokens per gather instruction (multiple of 128, divides RPC*n_t*128)
G = 384
# raw (non-Tile) pipeline with hand-placed semaphores: avoids the Tile
# preamble (~7us) and exit drain/barrier (~11us)
RAW = True
SINGLE_PACKET = True

TRACE = False
LAST = {}

_BUILD_CACHE = {}


def _pick_g(total_slots):
    for g in (G, 512, 256, 128):
        if total_slots % g == 0:
            return g
    return 128


def _windows(total_slots):
    """Tapered gather-window sizes: small first windows get the tensor engine
    started early; small last windows shorten the drain tail."""
    assert total_slots % P == 0
    if total_slots >= 6 * 384 and (total_slots - 768) % 384 == 0:
        k = (total_slots - 768) // 384
        return [128, 256] + [384] * k + [256, 128]
    g = _pick_g(total_slots)
    return [g] * (total_slots // g)


def _build_raw(n_t):
    """Raw Bacc pipeline (no TileContext) for one core's shard.

    Streams: sync loads consts + per-window lo chunks (HWDGE); gpsimd loads
    the mlp ucode then runs the transposed hi gathers back-to-back into
    distinct buffers; PE runs each window's matmuls once its gather + lo
    chunk land, recycling 8 PSUM banks guarded by the DVE's add_sem; DVE
    folds bias; sync writes each window's packed output as it completes.
    """
    from contextlib import ExitStack

    import concourse.mybir as mybir
    from concourse import bacc, library_config

    n_tiles = RPC * n_t
    wins = _windows(n_tiles * P)
    n_w = len(wins)
    starts = [sum(wins[:g]) for g in range(n_w)]  # slot offsets
    n_rows_pad = RPC * S + 1
    icols = sum(w // 16 for w in wins)

    nc = bacc.Bacc(
        "TRN2",
        debug=False,
        enable_asserts=False,
        num_devices=N_CORES,
        num_swdge_queues=1,
    )
    bf16 = mybir.dt.bfloat16
    f32 = mybir.dt.float32

    emb_hi = nc.dram_tensor("emb_hi", [n_rows_pad, D], bf16, kind="ExternalInput")
    lo_packed = nc.dram_tensor(
        "lo_packed", [P * DC * n_tiles * P], bf16, kind="ExternalInput")
    idx = nc.dram_tensor("idx", [P, icols], mybir.dt.int16, kind="ExternalInput")
    wt = nc.dram_tensor("wt", [P, DC, 2 * L], bf16, kind="ExternalInput")
    bmask = nc.dram_tensor("bmask", [P, n_tiles, L], f32, kind="ExternalInput")
    out = nc.dram_tensor("out", [n_tiles * P, L], f32, kind="ExternalOutput")

    PB = 8  # PSUM banks cycled

    with ExitStack() as st:
        idx_t = st.enter_context(
            nc.sbuf_tensor("idx_t", [P, icols], mybir.dt.int16))
        wt_t = st.enter_context(nc.sbuf_tensor("wt_t", [P, DC, 2 * L], bf16))
        bm_t = st.enter_context(nc.sbuf_tensor("bm_t", [P, n_tiles, L], f32))
        lo_t = st.enter_context(nc.sbuf_tensor("lo_t", [P, DC, n_tiles * P], bf16))
        out_t = st.enter_context(nc.sbuf_tensor("out_t", [P, n_tiles, L], f32))
        ghi = [st.enter_context(
            nc.sbuf_tensor(f"ghi{g}", [P, DC, wins[g]], bf16))
            for g in range(n_w)]
        pss = [st.enter_context(
            nc.psum_tensor(f"ps{b}", [P, 2 * L], f32)) for b in range(PB)]
        i_sem = st.enter_context(nc.semaphore("i_sem"))
        w_sem = st.enter_context(nc.semaphore("w_sem"))
        b_sem = st.enter_context(nc.semaphore("b_sem"))
        gs = [st.enter_context(nc.semaphore(f"gs{g}")) for g in range(n_w)]
        ls = [st.enter_context(nc.semaphore(f"ls{g}")) for g in range(n_w)]
        mm_sem = st.enter_context(nc.semaphore("mm_sem"))
        add_sem = st.enter_context(nc.semaphore("add_sem"))
        aux_sem = st.enter_context(nc.semaphore("aux_sem"))
        d_sem = st.enter_context(nc.semaphore("d_sem"))
        out_sem = st.enter_context(nc.semaphore("out_sem"))

        out_view = out.ap().rearrange("(i p) l -> p i l", p=P)

        # --- sync: const + lo loads (per-window [p, c, t] blocks: one big
        # --- contiguous descriptor per partition per window) ---
        nc.sync.dma_start(out=idx_t[:], in_=idx.ap()).then_inc(i_sem, 16)
        nc.sync.dma_start(out=wt_t[:], in_=wt.ap()).then_inc(w_sem, 16)
        for g in range(n_w):
            w = wins[g]
            ts_g = slice(starts[g], starts[g] + w)
            o_flat = P * DC * starts[g]
            src = lo_packed.ap()[o_flat:o_flat + P * DC * w].rearrange(
                "(p c t) -> p c t", p=P, c=DC)
            nc.sync.dma_start(out=lo_t[:, :, ts_g], in_=src).then_inc(ls[g], 16)
            if g == 1:
                nc.sync.dma_start(out=bm_t[:], in_=bmask.ap()).then_inc(b_sem, 16)

        # --- gpsimd: ucode library + transposed hi gathers ---
        nc.gpsimd.load_library(library_config.mlp)
        nc.gpsimd.wait_ge(i_sem, 16)  # idx_t loaded
        ic = 0
        for g in range(n_w):
            gc = wins[g] // 16
            nc.gpsimd.dma_gather(
                out_ap=ghi[g][:],
                in_ap=emb_hi.ap(),
                idxs_ap=idx_t[:, ic:ic + gc],
                num_idxs=wins[g],
                num_idxs_reg=wins[g],
                elem_size=D,
                transpose=True,
                queue_num=0,
                single_packet=SINGLE_PACKET,
            ).then_inc(gs[g], 16)
            ic += gc

        # --- tensor: per window, per 128-token tile ---
        nc.tensor.wait_ge(w_sem, 16)
        for g in range(n_w):
            nc.tensor.wait_ge(gs[g], 16)
            nc.tensor.wait_ge(ls[g], 16)
            for m in range(wins[g] // P):
                i = starts[g] // P + m
                if i >= PB:
                    nc.tensor.wait_ge(add_sem, i - PB + 1)
                ps = pss[i % PB]
                ms = slice(m * P, (m + 1) * P)
                ts_ = slice(i * P, (i + 1) * P)
                for c in range(DC):
                    nc.tensor.matmul(
                        out=ps[:, :],
                        lhsT=ghi[g][:, c, ms],
                        rhs=wt_t[:, c, :],
                        start=(c == 0),
                        stop=False,
                        skip_group_check=True,
                    )
                    mm = nc.tensor.matmul(
                        out=ps[:, 0:L],
                        lhsT=lo_t[:, c, ts_],
                        rhs=wt_t[:, c, 0:L],
                        start=False,
                        stop=(c == DC - 1),
                        skip_group_check=True,
                    )
                mm.then_inc(mm_sem, 1)

        # --- vector: bias fold + half-merge; drain per window so sync can
        # --- stream the output out behind us
        nc.vector.wait_ge(b_sem, 16)  # bm_t loaded
        for g in range(n_w):
            for m in range(wins[g] // P):
                i = starts[g] // P + m
                nc.vector.wait_ge(mm_sem, i + 1)
                ps = pss[i % PB]
                nc.vector.tensor_add(
                    out=out_t[:, i, :], in0=ps[:, 0:L], in1=bm_t[:, i, :]
                ).then_inc(aux_sem, 1)
                # same-engine RAW on out_t needs a completion wait
                nc.vector.wait_ge(aux_sem, i + 1)
                nc.vector.tensor_add(
                    out=out_t[:, i, :], in0=out_t[:, i, :], in1=ps[:, L:2 * L]
                ).then_inc(add_sem, 1)
            nc.vector.drain().then_inc(d_sem, 1)

        # --- sync: per-window packed output writeback ---
        done = 0
        for g in range(n_w):
            t1 = (starts[g] + wins[g]) // P
            nc.sync.wait_ge(d_sem, g + 1)
            nc.sync.dma_start(
                out=out_view[:, done:t1, :], in_=out_t[:, done:t1, :]
            ).then_inc(out_sem, 16)
            done = t1
        nc.sync.wait_ge(out_sem, 16 * n_w)

    nc.compile()
    return nc


def _get_nc(n_t):
    if n_t not in _BUILD_CACHE:
        _BUILD_CACHE[n_t] = _build_raw(n_t)
    return _BUILD_CACHE[n_t]


def _prep_inputs(embedding, mask, W, b):
    import ml_dtypes

    bf16 = ml_dtypes.bfloat16
    lens = mask.astype(np.int64).sum(axis=1)
    max_tok = int(lens.max())
    n_t = (max_tok + P - 1) // P
    n_tiles = RPC * n_t
    wins = _windows(n_tiles * P)
    npad = RPC * S  # index of the zero row

    emb2 = np.ascontiguousarray(embedding.reshape(B * S, D))
    ehi = emb2.astype(bf16)
    elo = (emb2 - ehi.astype(np.float32)).astype(bf16)

    Whi = W.astype(bf16)
    Wlo = (W - Whi.astype(np.float32)).astype(bf16)
    wt_host = np.empty((P, DC, 2 * L), dtype=bf16)
    wt_host[:, :, :L] = Whi.T.reshape(DC, P, L).transpose(1, 0, 2)
    wt_host[:, :, L:] = Wlo.T.reshape(DC, P, L).transpose(1, 0, 2)

    zrow = np.zeros((1, D), dtype=bf16)
    in_maps = []
    for k in range(N_CORES):
        r0 = k * RPC * S
        ehi_k = np.concatenate([ehi[r0:r0 + RPC * S], zrow], axis=0)
        elo_k = np.concatenate([elo[r0:r0 + RPC * S], zrow], axis=0)

        vals = np.full((n_tiles * P,), npad, dtype=np.int16)
        for r in range(RPC):
            pos = np.nonzero(mask[k * RPC + r])[0]
            lr = len(pos)
            o = r * n_t * P
            vals[o:o + lr] = (pos + r * S).astype(np.int16)
        valid = vals != npad
        # lo correction stream: host-packed + transposed, one contiguous
        # [P, DC, w] block per gather window
        packed = elo_k[vals].reshape(-1, DC, P)  # [slots, c, p]
        blocks, o = [], 0
        for w in wins:
            blocks.append(packed[o:o + w].transpose(2, 1, 0).ravel())
            o += w
        lo_packed = np.ascontiguousarray(np.concatenate(blocks))
        # SWDGE idx wrap per window: slot j -> partition j%16, col j//16,
        # replicated into every 16-partition group (each Q7 cpu of the serving
        # queue reads its own group).
        blocks, o = [], 0
        for w in wins:
            blocks.append(vals[o:o + w].reshape(w // 16, 16).T)
            o += w
        idx16 = np.concatenate(blocks, axis=1)
        idx_host = np.tile(idx16, (P // 16, 1))
        bm_host = (
            valid.reshape(n_tiles, P).T[:, :, None].astype(np.float32)
            * b.astype(np.float32)[None, None, :]
        )
        in_maps.append(
            dict(emb_hi=ehi_k, lo_packed=lo_packed, idx=idx_host,
                 wt=wt_host, bmask=np.ascontiguousarray(bm_host))
        )
    return in_maps, lens, max_tok, n_t


def kernel(embedding, mask, W, b):
    from concourse.bass_utils import run_bass_kernel_spmd

    embedding = np.asarray(embedding, dtype=np.float32)
    mask = np.asarray(mask)
    W = np.asarray(W, dtype=np.float32)
    b = np.asarray(b, dtype=np.float32)

    in_maps, lens, max_tok, n_t = _prep_inputs(embedding, mask, W, b)
    nc = _get_nc(n_t)

    res = run_bass_kernel_spmd(
        nc, in_maps, core_ids=list(range(N_CORES)), trace=TRACE
    )
    LAST["results"] = res

    outs = [r["out"].reshape(RPC, n_t * P, L) for r in res.results]
    tok_logits = np.concatenate(outs, axis=0)[:, :max_tok, :].astype(np.float32)
    pad_mask = np.arange(max_tok)[None, :] < lens[:, None]
    return tok_logits, pad_mask

